# revision 40
# baseline (speedup 1.0000x reference)
"""Dense dot-product attention (B=32, S=2048, D=128, fp32) on 8 TRN2 cores.

Sharding: batch dim B=32 split across 8 cores (4 batches/core); each core
computes full S x S attention for its batches independently (no collectives).

Host pre-converts Q,K,V to bf16 (ample accuracy headroom vs the 2e-2 gate;
bf16 is the fast PE dtype here — fp16 runs at half rate on this platform and
fp8 PV fails the accuracy gate) so the device does zero input casting and
input DMA halves. V is host-relayouted to [P, NJ, D] so its DMA is
contiguous per partition. Per-core kernel, per batch ("S^T layout", k on
partitions):
  for each q-phase (1024 wide), for each k-chunk j (16 x 128):
    S^T_j = Kt_j.T @ Qt[:, phase]      (PE, bf16 -> PSUM fp32)
    P^T_j = exp(scale * S^T_j)         (ACT, PSUM -> SBUF bf16)
    acc  += P^T_j                      (DVE bf16, 2x rate)
    O^T  += V_j.T @ P^T_j              (PE, PSUM fp32 [128d, q])
  drain: O^T -> SBUF fast (DVE) to free the single o_ps buffer;
  l = partition_all_reduce(acc) (GPSIMD, lands on all partitions);
  1/l (DVE fast reciprocal); O^T * (1/l) -> bf16 -> DMA out (host upcasts).

Schedule notes (from HW ablations; no trace hook on this platform):
- Default order is "jh": (batch, j) outer with both q-phases inner, so all
  4 scores matmuls of a j share kt_j and all 4 PV matmuls share v_j
  adjacently (helps the PE LDWEIGHTS pull-ahead); both phases' O^T
  accumulators live concurrently in PSUM (s_ps 2 + o_ps 2 = 8 banks).
- Platform walls (measured): PE p-states at ~1.2 GHz saturated; ACT PSUM
  reads starve to ~0.6 elem/cycle under a concurrent PE matmul stream.
  Period = max(PE cycles, ~2x starved exp) ~= 2.2us/tile-iteration.
- Scores for the next j-pair are emitted before the exp-blocked PV matmuls
  (PE is in-order). Drains/normalization/DMA and the GPSIMD partition
  reduction are fully hidden (ablation-verified).
"""

import sys

if "/opt/trn_rl_repo" not in sys.path:
    sys.path.insert(0, "/opt/trn_rl_repo")

import numpy as np

import concourse.bacc as bacc
import concourse.mybir as mybir
import concourse.tile as tile
from concourse import bass_utils
from concourse import bass_isa

N_CORES = 8
B = 32
S = 2048
D = 128
P = 128
BPC = B // N_CORES          # batches per core = 4
NJ = S // P                 # 16 k-chunks of 128
QH = 1024                   # q-phase width
NPH = S // QH               # 2 phases
NC_ = 512                   # matmul moving-operand chunk
SCALE = 1.0 / float(np.sqrt(D))

f32 = mybir.dt.float32
f16 = mybir.dt.bfloat16  # bf16: PE double-pumps bf16 (fp16 runs half-rate)
EXP = mybir.ActivationFunctionType.Exp


DEFAULT_ORDER = "jh"


def build(repeat=1, variant="full", s_bufs=2, o_bufs=2, order=None):
    """repeat>1 duplicates the whole per-core workload (same inputs/outputs)
    back-to-back inside one NEFF — used only for differential wall-clock
    timing of the hardware kernel (host/dispatch overhead cancels).

    variant: timing-only ablations ("full", "nodrain", "noadd", "nopv",
    "exponly", "noexp") — all but "full" produce wrong outputs and exist to
    locate the bottleneck engine on HW (no trace hook available)."""
    nc = bacc.Bacc("TRN2", target_bir_lowering=False, debug=False)

    Qtd = nc.dram_tensor("Qt", [BPC, D, S], f16, kind="ExternalInput")
    Ktd = nc.dram_tensor("Kt", [BPC, D, S], f16, kind="ExternalInput")
    Vd = nc.dram_tensor("V_p", [BPC, P, NJ, D], f16, kind="ExternalInput")
    Otd = nc.dram_tensor("Ot2", [BPC, D, S], f16, kind="ExternalOutput")

    with tile.TileContext(nc) as tc:
        with (
            tc.tile_pool(name="inp", bufs=3) as in_pool,
            tc.tile_pool(name="pt", bufs=8) as pt_pool,
            tc.tile_pool(name="misc", bufs=2) as misc_pool,
            tc.tile_pool(name="ot", bufs=2) as ot_pool,
            tc.tile_pool(name="acc", bufs=4) as acc_pool,
            tc.tile_pool(name="s_ps", bufs=s_bufs, space="PSUM") as s_pool,
            tc.tile_pool(name="o_ps", bufs=o_bufs, space="PSUM") as o_pool,
        ):
            inputs = {}
            NB = BPC * repeat

            def load_batch(bi):
                b = bi % BPC
                qt = in_pool.tile([P, S], f16, tag="qt")
                kt = in_pool.tile([P, S], f16, tag="kt")
                v_r = in_pool.tile([P, NJ, D], f16, tag="v_r")
                v_src = Vd[b]
                # head chunks first so compute can start early
                nc.sync.dma_start(kt[:, :256], Ktd[b, :, :256])
                nc.sync.dma_start(qt[:, :QH], Qtd[b, :, :QH])
                nc.sync.dma_start(v_r[:, :NJ // 2], v_src[:, :NJ // 2])
                nc.sync.dma_start(kt[:, 256:], Ktd[b, :, 256:])
                nc.sync.dma_start(qt[:, QH:], Qtd[b, :, QH:])
                nc.sync.dma_start(v_r[:, NJ // 2:], v_src[:, NJ // 2:])
                inputs[bi] = (qt, kt, v_r)

            load_batch(0)

            iters = [
                (bi, h, j)
                for bi in range(NB)
                for h in range(NPH)
                for j in range(NJ)
            ]
            T = len(iters)

            if order is None:
                order = DEFAULT_ORDER
            nc_w = NC_ if variant != "mm256" else 256
            do_exp = variant not in ("noexp", "mmonly", "mm256")
            exp_fixed = variant in ("expfixed", "accsc")
            acc_scores = variant == "accsc"
            do_add = variant in ("full", "nodrain", "nopv", "noexp")
            do_pv = variant in ("full", "nodrain", "noadd", "noexp")
            do_drain = variant == "full"

            def emit_scores(t):
                bi, h, j = iters[t]
                qt, kt, _ = inputs[bi]
                s_ps = s_pool.tile([P, QH], f32, tag="s")
                st = (t < 3) if acc_scores else True
                for c in range(QH // nc_w):
                    nc.tensor.matmul(
                        s_ps[:, c * nc_w:(c + 1) * nc_w],
                        kt[:, j * P:(j + 1) * P],
                        qt[:, h * QH + c * nc_w: h * QH + (c + 1) * nc_w],
                        start=st, stop=True,
                    )
                return s_ps

            if order in ("jh", "jhd", "jh2"):
                # (batch, j) outer, both q-phases inner: all 4 scores
                # matmuls of a pair share kt_j and all 4 PV matmuls share
                # v_j adjacently, giving the PE's LDWEIGHTS pull-ahead /
                # background weight buffer its best shot. Both phases'
                # O^T accumulators live concurrently (o_pool bufs=2).
                # "jhd" additionally defers PV emission by one pair: every
                # PV then consumes a pt tile whose exp finished a full pair
                # earlier (PE never waits on ACT mid-stream), and the batch-
                # boundary drain chain gets a whole pair window to free the
                # o_ps buffers before the next batch's first PV.
                defer_pv = order == "jhd"
                # "jh2": interleave PE emission per phase within a pair —
                # [scores-h0, pv-h0, scores-h1, pv-h1] — so the ready pv-h0
                # isn't queued behind scores-h1 (which waits on exp-h1) in
                # the in-order PE; fills the exp-h1 window with PV work.
                interleave = order == "jh2"
                pairs = [(bi, j) for bi in range(NB) for j in range(NJ)]

                def emit_scores_h(pi, h):
                    bi, j = pairs[pi]
                    qt, kt, _ = inputs[bi]
                    s_ps = s_pool.tile([P, QH], f32, tag="s", name=f"s{h}")
                    for c in range(QH // NC_):
                        nc.tensor.matmul(
                            s_ps[:, c * NC_:(c + 1) * NC_],
                            kt[:, j * P:(j + 1) * P],
                            qt[:, h * QH + c * NC_:
                               h * QH + (c + 1) * NC_],
                            start=True, stop=True,
                        )
                    return s_ps

                def emit_scores_pair(pi):
                    return [emit_scores_h(pi, h) for h in range(NPH)]

                o2 = None

                def alloc_o2():
                    nonlocal o2
                    o2 = [o_pool.tile([P, QH], f32, tag="o",
                                      name=f"o{h}") for h in range(NPH)]

                def emit_pv_h(bi, j, pt, h):
                    for c in range(QH // NC_):
                        nc.tensor.matmul(
                            o2[h][:, c * NC_:(c + 1) * NC_],
                            inputs[bi][2][:, j, :],
                            pt[:, c * NC_:(c + 1) * NC_],
                            start=(j == 0), stop=(j == NJ - 1),
                        )

                def emit_drains(bi, accs):
                    b = bi % BPC
                    # free BOTH phases' PSUM accumulators first (DVE is
                    # in-order: a copy queued behind recip would wait on the
                    # GPSIMD all_reduce and stall the next batch's PV)
                    o_sbs, lsums = [], []
                    for h in range(NPH):
                        o_sb = ot_pool.tile([P, QH], f32, tag="o_sb",
                                            name=f"ob{h}")
                        nc.vector.tensor_copy(o_sb[:], o2[h][:])
                        o_sbs.append(o_sb)
                    for h in range(NPH):
                        lsum = misc_pool.tile([P, QH], f32, tag="lsum",
                                              name=f"ls{h}")
                        nc.gpsimd.partition_all_reduce(
                            lsum[:], accs[h][:], channels=P,
                            reduce_op=bass_isa.ReduceOp.add,
                        )
                        lsums.append(lsum)
                    for h in range(NPH):
                        recip = misc_pool.tile([P, QH], f32, tag="recip",
                                               name=f"rc{h}")
                        nc.vector.reciprocal_approx_fast(recip[:],
                                                         lsums[h][:])
                        ot = ot_pool.tile([P, QH], f16, tag="ot",
                                          name=f"otl{h}")
                        nc.vector.tensor_mul(ot[:], o_sbs[h][:], recip[:])
                        nc.sync.dma_start(
                            Otd[b, :, h * QH:(h + 1) * QH], ot[:])

                def emit_pv_and_drain(bi, j, pts, accs):
                    if j == 0:
                        alloc_o2()
                    for h in range(NPH):
                        emit_pv_h(bi, j, pts[h], h)
                    if j == NJ - 1:
                        emit_drains(bi, accs)

                sp_q = [emit_scores_pair(0)]
                acc2 = None
                backlog = None  # (bi, j, pts, acc2) pending PV emission
                for pi in range(len(pairs)):
                    bi, j = pairs[pi]
                    if j == 0:
                        acc2 = [acc_pool.tile([P, QH], f16, tag="acc",
                                              name=f"a{h}")
                                for h in range(NPH)]
                    s_pair = sp_q.pop(0)
                    pts = []
                    for h in range(NPH):
                        pt = pt_pool.tile([P, QH], f16, tag="pt",
                                          name=f"pt{h}")
                        nc.scalar.activation(pt[:], s_pair[h][:], EXP,
                                             scale=SCALE)
                        pts.append(pt)
                    if j == 2 and bi + 1 < NB:
                        load_batch(bi + 1)
                    # deferred PV is ready work — it must precede the next
                    # scores matmuls (which wait on the exps just issued) in
                    # the in-order PE queue
                    if defer_pv and backlog is not None:
                        emit_pv_and_drain(*backlog)

                    def emit_adds():
                        for h in range(NPH):
                            if j == 0:
                                nc.vector.tensor_copy(acc2[h][:], pts[h][:])
                            else:
                                nc.vector.tensor_add(acc2[h][:], acc2[h][:],
                                                     pts[h][:])

                    if interleave:
                        emit_adds()
                        if j == 0:
                            alloc_o2()
                        nt = []
                        for h in range(NPH):
                            if pi + 1 < len(pairs):
                                nt.append(emit_scores_h(pi + 1, h))
                            emit_pv_h(bi, j, pts[h], h)
                        if nt:
                            sp_q.append(nt)
                        if j == NJ - 1:
                            emit_drains(bi, acc2)
                        continue
                    # emission order here is load-bearing: scores for the
                    # next pair FIRST, then the DVE adds, then the PV
                    # matmuls — the measured-best schedule (adds-first and
                    # adds-last permutations each cost ~8-30% on HW).
                    if pi + 1 < len(pairs):
                        sp_q.append(emit_scores_pair(pi + 1))
                    emit_adds()
                    if defer_pv:
                        backlog = (bi, j, pts, acc2)
                    else:
                        emit_pv_and_drain(bi, j, pts, acc2)
                if backlog is not None:
                    emit_pv_and_drain(*backlog)

            if order != "jh":
                # software pipeline: scores run TWO iterations ahead and
                # are emitted BEFORE the (exp-blocked) PV matmuls. The PE
                # is in-order, so any instruction behind pv(t) can't start
                # until exp(t) lands; keeping scores 2 ahead means ACT
                # always has a ready tile while PE waits.
                emit_hj = True
            else:
                emit_hj = False
            s_q = [emit_scores(0), emit_scores(1)] if emit_hj else []
            o_ps = acc = None
            pt_const = None
            s_fixed = None
            if exp_fixed:
                s_fixed = o_pool.tile([P, QH], f32, tag="sfix")
                nc.vector.memset(s_fixed[:], 0.3)
            if not do_exp:
                pt_const = misc_pool.tile([P, QH], f16, tag="ptc")
                nc.vector.memset(pt_const[:], 0.01)
            for t in range(T if emit_hj else 0):
                bi, h, j = iters[t]
                b = bi % BPC
                if j == 0:
                    if not exp_fixed:
                        o_ps = o_pool.tile([P, QH], f32, tag="o")
                    acc = acc_pool.tile([P, QH], f16, tag="acc")
                s_ps = s_q.pop(0)
                if do_exp:
                    pt = pt_pool.tile([P, QH], f16, tag="pt")
                    nc.scalar.activation(
                        pt[:], (s_fixed if exp_fixed else s_ps)[:], EXP,
                        scale=SCALE)
                else:
                    pt = pt_const
                # prefetch the next batch's inputs a full batch ahead
                if h == 0 and j == 2 and bi + 1 < NB:
                    load_batch(bi + 1)
                if t + 2 < T:
                    s_q.append(emit_scores(t + 2))
                # row sums: accumulate exp tiles on the DVE (j-partials) in
                # bf16 (2x rate); cross-partition reduction once per phase
                # on GPSIMD.
                if do_add:
                    if j == 0:
                        nc.vector.tensor_copy(acc[:], pt[:])
                    else:
                        nc.vector.tensor_add(acc[:], acc[:], pt[:])
                if do_pv:
                    for c in range(QH // NC_):
                        nc.tensor.matmul(
                            o_ps[:, c * NC_:(c + 1) * NC_],
                            inputs[bi][2][:, j, :],
                            pt[:, c * NC_:(c + 1) * NC_],
                            start=(j == 0), stop=(j == NJ - 1),
                        )
                if do_drain and j == NJ - 1:
                    # drain o_ps to SBUF immediately (DVE) so the next
                    # phase's first PV matmul isn't blocked on the
                    # normalization chain; normalize from SBUF off-path.
                    o_sb = ot_pool.tile([P, QH], f32, tag="o_sb")
                    nc.vector.tensor_copy(o_sb[:], o_ps[:])
                    # softmax denominators: sum acc across partitions on the
                    # (otherwise idle) GPSIMD; result lands on all partitions
                    lsum = misc_pool.tile([P, QH], f32, tag="lsum")
                    nc.gpsimd.partition_all_reduce(
                        lsum[:], acc[:], channels=P,
                        reduce_op=bass_isa.ReduceOp.add,
                    )
                    recip = misc_pool.tile([P, QH], f32, tag="recip")
                    nc.vector.reciprocal_approx_fast(recip[:], lsum[:])
                    ot = ot_pool.tile([P, QH], f16, tag="ot")
                    nc.vector.tensor_mul(ot[:], o_sb[:], recip[:])
                    nc.sync.dma_start(Otd[b, :, h * QH:(h + 1) * QH], ot[:])

    nc.compile()
    return nc


_nc_cache = None


def _get_nc():
    global _nc_cache
    if _nc_cache is None:
        _nc_cache = build()
    return _nc_cache


def make_in_maps(Q_p, K_p, V_p):
    """Host-side shard prep: transpose Q,K to [B, D, S], cast all to bf16,
    split across cores."""
    import ml_dtypes
    bf16 = ml_dtypes.bfloat16
    Qt = np.ascontiguousarray(
        np.asarray(Q_p, dtype=np.float32).transpose(0, 2, 1)
    ).astype(bf16)
    Kt = np.ascontiguousarray(
        np.asarray(K_p, dtype=np.float32).transpose(0, 2, 1)
    ).astype(bf16)
    V = np.asarray(V_p, dtype=np.float32).astype(bf16)
    # device-side layout: partition p holds rows {n*128+p}: [B, P, NJ, D]
    V = np.ascontiguousarray(
        V.reshape(B, S // P, P, D).transpose(0, 2, 1, 3)
    )
    return [
        {
            "Qt": Qt[c * BPC:(c + 1) * BPC],
            "Kt": Kt[c * BPC:(c + 1) * BPC],
            "V_p": V[c * BPC:(c + 1) * BPC],
        }
        for c in range(N_CORES)
    ]


def kernel(Q_p, K_p, V_p, trace=False):
    in_maps = make_in_maps(Q_p, K_p, V_p)
    nc = _get_nc()
    try:
        res = bass_utils.run_bass_kernel_spmd(
            nc, in_maps, core_ids=list(range(N_CORES)), trace=trace
        )
    except Exception:
        # shared terminals occasionally throw transient NRT errors; retry once
        import time as _time
        _time.sleep(5)
        res = bass_utils.run_bass_kernel_spmd(
            nc, in_maps, core_ids=list(range(N_CORES)), trace=trace
        )
    out = np.empty((B, S, D), dtype=np.float32)
    for c in range(N_CORES):
        ot = res.results[c]["Ot2"]  # [BPC, D, S] bf16
        out[c * BPC:(c + 1) * BPC] = ot.transpose(0, 2, 1).astype(np.float32)
    if trace:
        kernel.last_exec_time_ns = res.exec_time_ns
        kernel.last_results = res
    return out


# revision 41
# speedup vs baseline: 1.0390x; 1.0390x over previous
"""Dense dot-product attention (B=32, S=2048, D=128, fp32) on 8 TRN2 cores.

Sharding: batch dim B=32 split across 8 cores (4 batches/core); each core
computes full S x S attention for its batches independently (no collectives).

Host pre-converts Q,K,V to bf16 (ample accuracy headroom vs the 2e-2 gate;
bf16 is the fast PE dtype here — fp16 runs at half rate on this platform and
fp8 PV fails the accuracy gate) so the device does zero input casting and
input DMA halves. V is host-relayouted to [P, NJ, D] so its DMA is
contiguous per partition. Per-core kernel, per batch ("S^T layout", k on
partitions):
  for each q-phase (1024 wide), for each k-chunk j (16 x 128):
    S^T_j = Kt_j.T @ Qt[:, phase]      (PE, bf16 -> PSUM fp32)
    P^T_j = exp(scale * S^T_j)         (ACT, PSUM -> SBUF bf16)
    acc  += P^T_j                      (DVE bf16, 2x rate)
    O^T  += V_j.T @ P^T_j              (PE, PSUM fp32 [128d, q])
  drain: O^T -> SBUF fast (DVE) to free the single o_ps buffer;
  l = partition_all_reduce(acc) (GPSIMD, lands on all partitions);
  1/l (DVE fast reciprocal); O^T * (1/l) -> bf16 -> DMA out (host upcasts).

Schedule notes (from HW ablations; no trace hook on this platform):
- Default order is "jh": (batch, j) outer with both q-phases inner, so all
  4 scores matmuls of a j share kt_j and all 4 PV matmuls share v_j
  adjacently (helps the PE LDWEIGHTS pull-ahead); both phases' O^T
  accumulators live concurrently in PSUM (s_ps 2 + o_ps 2 = 8 banks).
- Platform walls (measured): PE p-states at ~1.2 GHz saturated; ACT PSUM
  reads starve to ~0.6 elem/cycle under a concurrent PE matmul stream.
  Period = max(PE cycles, ~2x starved exp) ~= 2.2us/tile-iteration.
- Scores for the next j-pair are emitted before the exp-blocked PV matmuls
  (PE is in-order). Drains/normalization/DMA and the GPSIMD partition
  reduction are fully hidden (ablation-verified).
"""

import sys

if "/opt/trn_rl_repo" not in sys.path:
    sys.path.insert(0, "/opt/trn_rl_repo")

import numpy as np

import concourse.bacc as bacc
import concourse.mybir as mybir
import concourse.tile as tile
from concourse import bass_utils
from concourse import bass_isa

N_CORES = 8
B = 32
S = 2048
D = 128
P = 128
BPC = B // N_CORES          # batches per core = 4
NJ = S // P                 # 16 k-chunks of 128
QH = 1024                   # q-phase width
NPH = S // QH               # 2 phases
NC_ = 512                   # matmul moving-operand chunk
SCALE = 1.0 / float(np.sqrt(D))

f32 = mybir.dt.float32
f16 = mybir.dt.bfloat16  # bf16: PE double-pumps bf16 (fp16 runs half-rate)
EXP = mybir.ActivationFunctionType.Exp


DEFAULT_ORDER = "jh"


def build(repeat=1, variant="full", s_bufs=2, o_bufs=2, order=None):
    """repeat>1 duplicates the whole per-core workload (same inputs/outputs)
    back-to-back inside one NEFF — used only for differential wall-clock
    timing of the hardware kernel (host/dispatch overhead cancels).

    variant: timing-only ablations ("full", "nodrain", "noadd", "nopv",
    "exponly", "noexp") — all but "full" produce wrong outputs and exist to
    locate the bottleneck engine on HW (no trace hook available)."""
    nc = bacc.Bacc("TRN2", target_bir_lowering=False, debug=False)

    Qtd = nc.dram_tensor("Qt", [BPC, D, S], f16, kind="ExternalInput")
    Ktd = nc.dram_tensor("Kt", [BPC, D, S], f16, kind="ExternalInput")
    Vd = nc.dram_tensor("V_p", [BPC, P, NJ, D], f16, kind="ExternalInput")
    Otd = nc.dram_tensor("Ot2", [BPC, D, S], f16, kind="ExternalOutput")

    with tile.TileContext(nc) as tc:
        with (
            tc.tile_pool(name="inp", bufs=3) as in_pool,
            tc.tile_pool(name="pt", bufs=8) as pt_pool,
            tc.tile_pool(name="misc", bufs=2) as misc_pool,
            tc.tile_pool(name="ot", bufs=2) as ot_pool,
            tc.tile_pool(name="acc", bufs=4) as acc_pool,
            tc.tile_pool(name="s_ps", bufs=s_bufs, space="PSUM") as s_pool,
            tc.tile_pool(name="o_ps", bufs=o_bufs, space="PSUM") as o_pool,
        ):
            inputs = {}
            NB = BPC * repeat

            def load_batch(bi):
                b = bi % BPC
                qt = in_pool.tile([P, S], f16, tag="qt")
                kt = in_pool.tile([P, S], f16, tag="kt")
                v_r = in_pool.tile([P, NJ, D], f16, tag="v_r")
                v_src = Vd[b]
                # head chunks first so compute can start early
                nc.sync.dma_start(kt[:, :256], Ktd[b, :, :256])
                nc.sync.dma_start(qt[:, :QH], Qtd[b, :, :QH])
                nc.sync.dma_start(v_r[:, :NJ // 2], v_src[:, :NJ // 2])
                nc.sync.dma_start(kt[:, 256:], Ktd[b, :, 256:])
                nc.sync.dma_start(qt[:, QH:], Qtd[b, :, QH:])
                nc.sync.dma_start(v_r[:, NJ // 2:], v_src[:, NJ // 2:])
                inputs[bi] = (qt, kt, v_r)

            load_batch(0)

            iters = [
                (bi, h, j)
                for bi in range(NB)
                for h in range(NPH)
                for j in range(NJ)
            ]
            T = len(iters)

            if order is None:
                order = DEFAULT_ORDER
            nc_w = NC_ if variant != "mm256" else 256
            do_exp = variant not in ("noexp", "mmonly", "mm256")
            exp_fixed = variant in ("expfixed", "accsc")
            acc_scores = variant == "accsc"
            do_add = variant in ("full", "nodrain", "nopv", "noexp")
            do_pv = variant in ("full", "nodrain", "noadd", "noexp")
            do_drain = variant == "full"

            def emit_scores(t):
                bi, h, j = iters[t]
                qt, kt, _ = inputs[bi]
                s_ps = s_pool.tile([P, QH], f32, tag="s")
                st = (t < 3) if acc_scores else True
                for c in range(QH // nc_w):
                    nc.tensor.matmul(
                        s_ps[:, c * nc_w:(c + 1) * nc_w],
                        kt[:, j * P:(j + 1) * P],
                        qt[:, h * QH + c * nc_w: h * QH + (c + 1) * nc_w],
                        start=st, stop=True,
                    )
                return s_ps

            if order in ("jh", "jhd", "jh2"):
                # (batch, j) outer, both q-phases inner: all 4 scores
                # matmuls of a pair share kt_j and all 4 PV matmuls share
                # v_j adjacently, giving the PE's LDWEIGHTS pull-ahead /
                # background weight buffer its best shot. Both phases'
                # O^T accumulators live concurrently (o_pool bufs=2).
                # "jhd" additionally defers PV emission by one pair: every
                # PV then consumes a pt tile whose exp finished a full pair
                # earlier (PE never waits on ACT mid-stream), and the batch-
                # boundary drain chain gets a whole pair window to free the
                # o_ps buffers before the next batch's first PV.
                defer_pv = order == "jhd"
                # "jh2": interleave PE emission per phase within a pair —
                # [scores-h0, pv-h0, scores-h1, pv-h1] — so the ready pv-h0
                # isn't queued behind scores-h1 (which waits on exp-h1) in
                # the in-order PE; fills the exp-h1 window with PV work.
                interleave = order == "jh2"
                pairs = [(bi, j) for bi in range(NB) for j in range(NJ)]

                def emit_scores_h(pi, h):
                    bi, j = pairs[pi]
                    qt, kt, _ = inputs[bi]
                    s_ps = s_pool.tile([P, QH], f32, tag="s", name=f"s{h}")
                    for c in range(QH // NC_):
                        nc.tensor.matmul(
                            s_ps[:, c * NC_:(c + 1) * NC_],
                            kt[:, j * P:(j + 1) * P],
                            qt[:, h * QH + c * NC_:
                               h * QH + (c + 1) * NC_],
                            start=True, stop=True,
                        )
                    return s_ps

                def emit_scores_pair(pi):
                    return [emit_scores_h(pi, h) for h in range(NPH)]

                o2 = None

                def alloc_o2():
                    nonlocal o2
                    o2 = [o_pool.tile([P, QH], f32, tag="o",
                                      name=f"o{h}") for h in range(NPH)]

                def emit_pv_h(bi, j, pt, h):
                    for c in range(QH // NC_):
                        nc.tensor.matmul(
                            o2[h][:, c * NC_:(c + 1) * NC_],
                            inputs[bi][2][:, j, :],
                            pt[:, c * NC_:(c + 1) * NC_],
                            start=(j == 0), stop=(j == NJ - 1),
                        )

                def emit_drains(bi, accs):
                    b = bi % BPC
                    # per-phase grouped drain order is the measured best
                    # (copies-first reordering regressed ~5%, like every
                    # other emission-order deviation)
                    for h in range(NPH):
                        o_sb = ot_pool.tile([P, QH], f32, tag="o_sb",
                                            name=f"ob{h}")
                        nc.vector.tensor_copy(o_sb[:], o2[h][:])
                        lsum = misc_pool.tile([P, QH], f32, tag="lsum",
                                              name=f"ls{h}")
                        nc.gpsimd.partition_all_reduce(
                            lsum[:], accs[h][:], channels=P,
                            reduce_op=bass_isa.ReduceOp.add,
                        )
                        recip = misc_pool.tile([P, QH], f32, tag="recip",
                                               name=f"rc{h}")
                        nc.vector.reciprocal_approx_fast(recip[:], lsum[:])
                        ot = ot_pool.tile([P, QH], f16, tag="ot",
                                          name=f"otl{h}")
                        nc.vector.tensor_mul(ot[:], o_sb[:], recip[:])
                        nc.sync.dma_start(
                            Otd[b, :, h * QH:(h + 1) * QH], ot[:])

                def emit_pv_and_drain(bi, j, pts, accs):
                    if j == 0:
                        alloc_o2()
                    for h in range(NPH):
                        emit_pv_h(bi, j, pts[h], h)
                    if j == NJ - 1:
                        emit_drains(bi, accs)

                sp_q = [emit_scores_pair(0)]
                acc2 = None
                backlog = None  # (bi, j, pts, acc2) pending PV emission
                for pi in range(len(pairs)):
                    bi, j = pairs[pi]
                    if j == 0:
                        acc2 = [acc_pool.tile([P, QH], f16, tag="acc",
                                              name=f"a{h}")
                                for h in range(NPH)]
                    s_pair = sp_q.pop(0)
                    pts = []
                    for h in range(NPH):
                        pt = pt_pool.tile([P, QH], f16, tag="pt",
                                          name=f"pt{h}")
                        nc.scalar.activation(pt[:], s_pair[h][:], EXP,
                                             scale=SCALE)
                        pts.append(pt)
                    if j == 2 and bi + 1 < NB:
                        load_batch(bi + 1)
                    # deferred PV is ready work — it must precede the next
                    # scores matmuls (which wait on the exps just issued) in
                    # the in-order PE queue
                    if defer_pv and backlog is not None:
                        emit_pv_and_drain(*backlog)

                    def emit_adds():
                        for h in range(NPH):
                            if j == 0:
                                nc.vector.tensor_copy(acc2[h][:], pts[h][:])
                            else:
                                nc.vector.tensor_add(acc2[h][:], acc2[h][:],
                                                     pts[h][:])

                    if interleave:
                        emit_adds()
                        if j == 0:
                            alloc_o2()
                        nt = []
                        for h in range(NPH):
                            if pi + 1 < len(pairs):
                                nt.append(emit_scores_h(pi + 1, h))
                            emit_pv_h(bi, j, pts[h], h)
                        if nt:
                            sp_q.append(nt)
                        if j == NJ - 1:
                            emit_drains(bi, acc2)
                        continue
                    # emission order here is load-bearing: scores for the
                    # next pair FIRST, then the DVE adds, then the PV
                    # matmuls — the measured-best schedule (adds-first and
                    # adds-last permutations each cost ~8-30% on HW).
                    if pi + 1 < len(pairs):
                        sp_q.append(emit_scores_pair(pi + 1))
                    emit_adds()
                    if defer_pv:
                        backlog = (bi, j, pts, acc2)
                    else:
                        emit_pv_and_drain(bi, j, pts, acc2)
                if backlog is not None:
                    emit_pv_and_drain(*backlog)

            if order != "jh":
                # software pipeline: scores run TWO iterations ahead and
                # are emitted BEFORE the (exp-blocked) PV matmuls. The PE
                # is in-order, so any instruction behind pv(t) can't start
                # until exp(t) lands; keeping scores 2 ahead means ACT
                # always has a ready tile while PE waits.
                emit_hj = True
            else:
                emit_hj = False
            s_q = [emit_scores(0), emit_scores(1)] if emit_hj else []
            o_ps = acc = None
            pt_const = None
            s_fixed = None
            if exp_fixed:
                s_fixed = o_pool.tile([P, QH], f32, tag="sfix")
                nc.vector.memset(s_fixed[:], 0.3)
            if not do_exp:
                pt_const = misc_pool.tile([P, QH], f16, tag="ptc")
                nc.vector.memset(pt_const[:], 0.01)
            for t in range(T if emit_hj else 0):
                bi, h, j = iters[t]
                b = bi % BPC
                if j == 0:
                    if not exp_fixed:
                        o_ps = o_pool.tile([P, QH], f32, tag="o")
                    acc = acc_pool.tile([P, QH], f16, tag="acc")
                s_ps = s_q.pop(0)
                if do_exp:
                    pt = pt_pool.tile([P, QH], f16, tag="pt")
                    nc.scalar.activation(
                        pt[:], (s_fixed if exp_fixed else s_ps)[:], EXP,
                        scale=SCALE)
                else:
                    pt = pt_const
                # prefetch the next batch's inputs a full batch ahead
                if h == 0 and j == 2 and bi + 1 < NB:
                    load_batch(bi + 1)
                if t + 2 < T:
                    s_q.append(emit_scores(t + 2))
                # row sums: accumulate exp tiles on the DVE (j-partials) in
                # bf16 (2x rate); cross-partition reduction once per phase
                # on GPSIMD.
                if do_add:
                    if j == 0:
                        nc.vector.tensor_copy(acc[:], pt[:])
                    else:
                        nc.vector.tensor_add(acc[:], acc[:], pt[:])
                if do_pv:
                    for c in range(QH // NC_):
                        nc.tensor.matmul(
                            o_ps[:, c * NC_:(c + 1) * NC_],
                            inputs[bi][2][:, j, :],
                            pt[:, c * NC_:(c + 1) * NC_],
                            start=(j == 0), stop=(j == NJ - 1),
                        )
                if do_drain and j == NJ - 1:
                    # drain o_ps to SBUF immediately (DVE) so the next
                    # phase's first PV matmul isn't blocked on the
                    # normalization chain; normalize from SBUF off-path.
                    o_sb = ot_pool.tile([P, QH], f32, tag="o_sb")
                    nc.vector.tensor_copy(o_sb[:], o_ps[:])
                    # softmax denominators: sum acc across partitions on the
                    # (otherwise idle) GPSIMD; result lands on all partitions
                    lsum = misc_pool.tile([P, QH], f32, tag="lsum")
                    nc.gpsimd.partition_all_reduce(
                        lsum[:], acc[:], channels=P,
                        reduce_op=bass_isa.ReduceOp.add,
                    )
                    recip = misc_pool.tile([P, QH], f32, tag="recip")
                    nc.vector.reciprocal_approx_fast(recip[:], lsum[:])
                    ot = ot_pool.tile([P, QH], f16, tag="ot")
                    nc.vector.tensor_mul(ot[:], o_sb[:], recip[:])
                    nc.sync.dma_start(Otd[b, :, h * QH:(h + 1) * QH], ot[:])

    nc.compile()
    return nc


_nc_cache = None


def _get_nc():
    global _nc_cache
    if _nc_cache is None:
        _nc_cache = build()
    return _nc_cache


def make_in_maps(Q_p, K_p, V_p):
    """Host-side shard prep: transpose Q,K to [B, D, S], cast all to bf16,
    split across cores."""
    import ml_dtypes
    bf16 = ml_dtypes.bfloat16
    Qt = np.ascontiguousarray(
        np.asarray(Q_p, dtype=np.float32).transpose(0, 2, 1)
    ).astype(bf16)
    Kt = np.ascontiguousarray(
        np.asarray(K_p, dtype=np.float32).transpose(0, 2, 1)
    ).astype(bf16)
    V = np.asarray(V_p, dtype=np.float32).astype(bf16)
    # device-side layout: partition p holds rows {n*128+p}: [B, P, NJ, D]
    V = np.ascontiguousarray(
        V.reshape(B, S // P, P, D).transpose(0, 2, 1, 3)
    )
    return [
        {
            "Qt": Qt[c * BPC:(c + 1) * BPC],
            "Kt": Kt[c * BPC:(c + 1) * BPC],
            "V_p": V[c * BPC:(c + 1) * BPC],
        }
        for c in range(N_CORES)
    ]


def kernel(Q_p, K_p, V_p, trace=False):
    in_maps = make_in_maps(Q_p, K_p, V_p)
    nc = _get_nc()
    try:
        res = bass_utils.run_bass_kernel_spmd(
            nc, in_maps, core_ids=list(range(N_CORES)), trace=trace
        )
    except Exception:
        # shared terminals occasionally throw transient NRT errors; retry once
        import time as _time
        _time.sleep(5)
        res = bass_utils.run_bass_kernel_spmd(
            nc, in_maps, core_ids=list(range(N_CORES)), trace=trace
        )
    out = np.empty((B, S, D), dtype=np.float32)
    for c in range(N_CORES):
        ot = res.results[c]["Ot2"]  # [BPC, D, S] bf16
        out[c * BPC:(c + 1) * BPC] = ot.transpose(0, 2, 1).astype(np.float32)
    if trace:
        kernel.last_exec_time_ns = res.exec_time_ns
        kernel.last_results = res
    return out
